# revision 47
# baseline (speedup 1.0000x reference)
"""Multi-head self-attention (AdaptiveTemporalContrastEnhancement) on 8 TRN2 cores.

v3: ACT-paced flat pipeline + parallel startup DMAs + f32 denominator +
restructured tail. (v2 baseline: 210-218us, rel err 1.7547e-2.)

Key facts baked in:
- delta_c bias is uniform along the softmax axis -> softmax cancels it -> skipped.
- max |logit| ~ 1.9 -> softmax without max-subtraction; A=exp(S) fits fp8e4m3.
- V bias + output bias fold: out = A@(XWv^T)Wo^T + (Wo bv + bo).
- 1/sqrt(dh) folded into WQT/BQ host-side.
- Data parallel over 16 (b,t) slices: 2 slices/core, no collectives.
- Projections + S in bf16; PV in fp8 DoubleRow (2x contraction per matmul).
- fp8 projections are numerically dead (numsim: v_fp8 2.66e-2 > 2e-2 gate).
- Denominator chain in f32 (gather straight from PSUM): 1.754e-2 -> 1.636e-2.

Structure (the Tile scheduler is a greedy list scheduler; emission order is
priority):
- Startup DMAs spread across engine queues (sync/gpsimd/scalar/vector) so the
  QK(s0,et0,qh0) critical path isn't serialized behind 4.5MB on one queue.
- Pre-attention: QK(s0,et0,qh0) (8 mm) + V0 kv0,kv1 (8 mm) only.
- One flat 128-step loop over (slice, head-pair j, q-half, kv): emit S (2 mm)
  + exp per step; PV (2 DR mm) deferred LOOK=2 steps so the exp stream never
  waits on S and PE never parks on a PV ahead of ready S work.
- Remaining projections stream in as a deadline-ordered filler chain popped
  ~2-3/step (priorities only; the scheduler timing-fits them).
- Slice-1 groups ordered j=1,2,3,0 so the tail out-proj's closing dd=0
  accumulation is the only thing waiting on the final norm chain.
- Warm pool (small free-256 matmuls) emitted late: scheduler sprinkles them
  into PE idle gaps to hold the HAM clock gate at full p-state.
- Tail: _acc accumulates dd=1,2,3 for all (et,qh) during the last norm chain;
  closers add dd=0, bias, and DMA out per q-half.
"""

import os
import numpy as np
import ml_dtypes

B, T, N, D = 2, 8, 1024, 512
H, DH = 8, 64
P = 128
NDT = D // P          # 4 d-tiles
NKV = N // P          # 8 kv tiles
NPAIR = NKV // 2      # 4 kv pairs
NQH = N // 512        # 2 q halves
NCORES = 8
NSLICE = (B * T) // NCORES   # 2 slices per core
S_SCALE = float(1.0 / np.sqrt(DH))  # 0.125
LOOK = 4              # steps of PV deferral behind S/exp

_CACHE = {}


def _build_nc():
    import concourse.mybir as mybir
    from concourse import bacc
    from concourse.tile import TileContext
    import concourse.bass as bass

    f32, bf16 = mybir.dt.float32, mybir.dt.bfloat16
    f8 = mybir.dt.float8e4
    nc = bacc.Bacc("TRN2", target_bir_lowering=False, debug=False)

    XT = nc.dram_tensor("XT", [NSLICE, D, N], bf16, kind="ExternalInput")
    WQT = nc.dram_tensor("WQT", [D, D], bf16, kind="ExternalInput")
    WKT = nc.dram_tensor("WKT", [D, D], bf16, kind="ExternalInput")
    WVT = nc.dram_tensor("WVT", [D, D], bf16, kind="ExternalInput")
    WOT = nc.dram_tensor("WOT", [D, D], bf16, kind="ExternalInput")
    BQ = nc.dram_tensor("BQ", [NDT, P, 1], f32, kind="ExternalInput")
    BK = nc.dram_tensor("BK", [NDT, P, 1], f32, kind="ExternalInput")
    BO = nc.dram_tensor("BO", [NDT, P, 1], f32, kind="ExternalInput")
    OT = nc.dram_tensor("OT", [NSLICE, D, N], bf16, kind="ExternalOutput")

    Exp = mybir.ActivationFunctionType.Exp
    Mult = mybir.AluOpType.mult
    DR = mybir.MatmulPerfMode.DoubleRow

    with TileContext(nc) as tc:
        with (
            tc.tile_pool(name="wpool", bufs=1) as wpool,
            tc.tile_pool(name="xpool", bufs=2) as xpool,
            tc.tile_pool(name="qkpool", bufs=2) as qkpool,
            tc.tile_pool(name="vpool", bufs=2) as vpool,
            tc.tile_pool(name="apool", bufs=4) as apool,
            tc.tile_pool(name="zpool", bufs=2) as zpool,
            tc.tile_pool(name="rpool", bufs=2) as rpool,
            tc.tile_pool(name="opool", bufs=3) as opool,
            tc.tile_pool(name="drpool", bufs=4, space="DRAM") as drpool,
            tc.tile_pool(name="ps_s", bufs=2, space="PSUM") as ps_s,
            tc.tile_pool(name="ps_z", bufs=2, space="PSUM") as ps_z,
            tc.tile_pool(name="ps_c", bufs=1, space="PSUM") as ps_c,
        ):
            w_sb, b_sb = {}, {}

            def emit_w(name, dram, eng):
                t = wpool.tile([P, NDT * 512], bf16, tag=name, name=f"w_{name}")
                w_sb[name] = t
                eng.dma_start(
                    out=t[:, :].rearrange("p (dt e) -> p dt e", e=512),
                    in_=dram[:, :].rearrange("(dt p) e -> p dt e", p=P),
                )

            def emit_b(name, dram, eng):
                t = wpool.tile([P, NDT], f32, tag=name, name=f"b_{name}")
                b_sb[name] = t
                eng.dma_start(
                    out=t[:, :],
                    in_=dram[:, :, :].rearrange("et p one -> p (et one)"),
                )

            def load_x(s, eng):
                xt = xpool.tile([P, NDT * N], bf16, tag="xt", name=f"xt_{s}")
                eng.dma_start(
                    out=xt[:, :].rearrange("p (dt n) -> p dt n", n=N),
                    in_=XT[s].rearrange("(dt p) n -> p dt n", p=P),
                )
                return xt

            # startup loads: critical-path tensors first, one per queue
            # (DMA-capable engines: sync/SP, scalar/ACT, gpsimd/Pool)
            xt0 = load_x(0, nc.sync)
            emit_w("wq", WQT, nc.gpsimd)
            emit_b("bq", BQ, nc.scalar)
            emit_b("bk", BK, nc.scalar)
            emit_w("wk", WKT, nc.scalar)
            emit_w("wv", WVT, nc.gpsimd)
            xt1 = load_x(1, nc.sync)
            emit_w("wo", WOT, nc.scalar)
            emit_b("bo", BO, nc.scalar)

            _par = [0]

            def _fps(nm):
                # alternate filler psum between two 1-bank tiles so a new
                # filler sub-chunk never waits on the previous one's evac
                _par[0] ^= 1
                return ps_c.tile([P, 512], f32, tag=f"c{_par[0]}", name=nm)

            def gen_qk_half(s, et, qh, xt, qt, kt):
                """Filler: Q then K projection for one (e-tile, q-half)."""
                for dst, wname, bname in ((qt[et], "wq", "bq"),
                                          (kt[et], "wk", "bk")):
                    w = w_sb[wname]
                    ps = _fps(f"psc_{wname}_{s}_{et}_{qh}")
                    for dt_ in range(NDT):
                        nc.tensor.matmul(
                            ps,
                            lhsT=w[:, dt_ * 512 + et * P: dt_ * 512 + (et + 1) * P],
                            rhs=xt[:, dt_ * N + qh * 512: dt_ * N + qh * 512 + 512],
                            start=(dt_ == 0), stop=(dt_ == NDT - 1),
                        )
                        if dt_ == NDT - 1:
                            nc.vector.tensor_scalar_add(
                                dst[:, qh * 512: qh * 512 + 512],
                                ps, b_sb[bname][:, et:et + 1],
                            )
                        yield

            def gen_v_chunk(s, kv, xt, v_sb):
                """Filler: V projection + fp8 pad layout for one kv tile."""
                ps = _fps(f"ps_v_{s}_{kv}")
                for dt_ in range(NDT):
                    nc.tensor.matmul(
                        ps,
                        lhsT=xt[:, dt_ * N + kv * P: dt_ * N + (kv + 1) * P],
                        rhs=w_sb["wv"][:, dt_ * 512:(dt_ + 1) * 512],
                        start=(dt_ == 0), stop=(dt_ == NDT - 1),
                    )
                    if dt_ == NDT - 1:
                        vblk = v_sb[:, kv * 1024:(kv + 1) * 1024].rearrange(
                            "p (hp r) -> p hp r", r=256)
                        psh = ps.rearrange("p (hp c) -> p hp c", c=128)
                        nc.vector.tensor_copy(vblk[:, :, 0:64], psh[:, :, 0:64])
                        nc.vector.tensor_copy(vblk[:, :, 192:256], psh[:, :, 64:128])
                        if s == 0:
                            nc.vector.tensor_copy(
                                vb0[:, kv * 256: kv * 256 + 64], ps[:, 0:64])
                            nc.vector.tensor_copy(
                                vb0[:, kv * 256 + 192: kv * 256 + 256],
                                ps[:, 64:128])
                    yield

            def gen_op_chunk(s, et, zt):
                """Filler: out-projection for one e-tile of slice s."""
                o_sb = opool.tile([P, N], bf16, tag="o", name=f"o_{s}_{et}")
                for qh in range(NQH):
                    ps = _fps(f"psc_o_{s}_{et}_{qh}")
                    for dd in range(NDT):
                        nc.tensor.matmul(
                            ps,
                            lhsT=w_sb["wo"][:, dd * 512 + et * P: dd * 512 + (et + 1) * P],
                            rhs=zt[dd][:, qh * 512: qh * 512 + 512],
                            start=(dd == 0), stop=(dd == NDT - 1),
                        )
                        if dd == NDT - 1:
                            nc.vector.tensor_scalar_add(
                                o_sb[:, qh * 512:(qh + 1) * 512],
                                ps, b_sb["bo"][:, et:et + 1])
                            if qh == NQH - 1:
                                nc.sync.dma_start(
                                    out=OT[s, et * P:(et + 1) * P, :], in_=o_sb)
                        yield

            def v_pad_init(s):
                # all memsets on DVE: the gpsimd queue must stay clear for the
                # startup weight DMAs, and DVE is idle until the first bias add
                v_sb = vpool.tile([P, NKV * H * P], f8, tag="v", name=f"v_{s}")
                vz = v_sb.rearrange("p (b r) -> p b r", r=256)
                # big zero-pads on gpsimd (idle after its 2 weight DMAs):
                # on the DVE queue they delayed the first QK bias-adds and
                # pushed the first exp to ~28us
                nc.gpsimd.memset(vz[:, :, 65:128], 0.0)    # even-head pad
                nc.gpsimd.memset(vz[:, :, 129:192], 0.0)   # odd-head pad
                nc.vector.memset(vz[:, :, 64:65], 1.0)     # even-head ones col
                nc.vector.memset(vz[:, :, 128:129], 1.0)   # odd-head ones col
                return v_sb

            def alloc_attn(s):
                zun = zpool.tile([P, H * N], bf16, tag="zun", name=f"zun_{s}")
                zt = [zpool.tile([P, N], bf16, tag=f"zt{j}", name=f"zt_{s}_{j}")
                      for j in range(NDT)]
                return zun, zt

            # ---- attention step/PV/chain emitters ----
            def emit_S(s, j, qh, kv, qt, kt, sps):
                for p_ in range(2):
                    pb = 64 * p_
                    nc.tensor.matmul(
                        sps[:, p_ * 512:(p_ + 1) * 512],
                        lhsT=kt[j][pb:pb + 64, kv * P:(kv + 1) * P],
                        rhs=qt[j][pb:pb + 64, qh * 512: qh * 512 + 512],
                        start=True, stop=True,
                    )

            def emit_E(sps, at2, sl):
                at2v = at2[:, :].rearrange("p (h sl n) -> p h sl n", h=2, sl=2)
                nc.scalar.activation(
                    at2v[:, :, sl, :],
                    sps[:, :].rearrange("p (h n) -> p h n", h=2),
                    Exp)

            def emit_PV(s, j, qh, b, at2, v_sb, zs):
                # The DR weights AP (3D rearranged slice) is INVISIBLE to the
                # dependency tracker (verified: first PV's only deps were the
                # two exps — no V-copy/memset deps). Force the ordering with a
                # PE nop that reads the kv-pair's V region as a plain 2D
                # slice; in-order PE execution then covers the matmuls.
                dep = mybir.InstNoOp(
                    name=nc.get_next_instruction_name(),
                    text_hint="vdep",
                    ins=[nc.tensor.lower_ap(
                        v_sb[:, 2 * b * 1024:(2 * b + 2) * 1024])],
                )
                nc.tensor.add_instruction(dep)
                a3 = at2[:, :].rearrange("p (h r) -> p h r", h=2)
                v3 = v_sb.rearrange("p (kv r) -> p kv r", r=1024)
                for p_ in range(2):
                    h = 2 * j + p_
                    nc.tensor.matmul(
                        zs[p_],
                        lhsT=v3[:, 2 * b:2 * b + 2, h * P:(h + 1) * P],
                        rhs=a3[:, p_:p_ + 1, :].rearrange(
                            "p one (sl n) -> p (one sl) n", sl=2),
                        start=(b == 0), stop=(b == NPAIR - 1),
                        perf_mode=DR,
                    )

            def emit_PV_bf16(b, zs):
                # normal-mode PV for the first group: contraction 128 kv
                # tokens per matmul, 2 sl x 2 p_ per pair
                for sl in range(2):
                    kv = 2 * b + sl
                    for p_ in range(2):
                        nc.tensor.matmul(
                            zs[p_],
                            lhsT=vb0[:, kv * 256 + p_ * 128:
                                     kv * 256 + (p_ + 1) * 128],
                            rhs=at0[:, b * 2048 + p_ * 1024 + sl * 512:
                                    b * 2048 + p_ * 1024 + sl * 512 + 512],
                            start=(kv == 0), stop=(kv == NKV - 1),
                        )

            def emit_chain_a(s, j, qh, zs, zun, cell):
                """Chain part A (all PSUM readers): evac, den-row stage,
                compact gather, f32 reciprocal, DRAM bounce."""
                for p_ in range(2):
                    h = 2 * j + p_
                    nc.vector.tensor_copy(
                        zun[:, h * N + qh * 512: h * N + qh * 512 + 512],
                        zs[p_])
                dstage = rpool.tile([P, 1024], f32, tag="dstage",
                                    name=f"dstage_{s}_{j}_{qh}")
                dall = rpool.tile([16, 64], f32, tag="dall",
                                  name=f"dall_{s}_{j}_{qh}")
                dallr = rpool.tile([16, 64], f32, tag="dallr",
                                   name=f"dallr_{s}_{j}_{qh}")
                rdram = drpool.tile([16, 64], f32, tag="rdram",
                                    name=f"rdram_{s}_{j}_{qh}")
                for p_ in range(2):
                    h = 2 * j + p_
                    dr_ = 64 if h % 2 == 0 else 0
                    nc.vector.tensor_copy(
                        dstage[dr_:dr_ + 1, p_ * 512:(p_ + 1) * 512],
                        zs[p_][dr_:dr_ + 1, :])
                    nc.sync.dma_start(
                        out=dall[8 * p_:8 * p_ + 8, :],
                        in_=dstage[dr_:dr_ + 1, p_ * 512:(p_ + 1) * 512])
                # approx_fast is ~18 correct bits; base partition must be 0
                # (partition-64 custom-DVE ops return garbage, measured)
                nc.vector.reciprocal_approx_fast(out=dallr, in_=dall)
                nc.sync.dma_start(out=rdram[:, :], in_=dallr)
                cell['rdram'] = rdram

            def emit_chain_b(s, j, qh, zun, zt, cell):
                """Chain part B: read-back, broadcast, TT-mult. Split from A
                so the first group's B can be deferred past the empty-queue
                DMA write-visibility window without touching PSUM rotation."""
                rdram = cell['rdram']
                # read-back on the same queue: the bounce's completion sem
                # can fire before the DRAM write is visible to other rings
                # (first-execution trial-0 NaN); pull the write through
                # before the broadcast reads it.
                rjunk = rpool.tile([16, 64], f32, tag="rjunk",
                                   name=f"rjunk_{s}_{j}_{qh}")
                nc.sync.dma_start(out=rjunk[:, :], in_=rdram[:, :])
                rbcr = rpool.tile([P, 512], f32, tag="rbcr",
                                  name=f"rbcr_{s}_{j}_{qh}")
                # bcast rides the SAME queue as the bounce: the raw-AP read
                # below is not WAR-tracked against the next tenant's bounce
                # (verified in the built program), so queue order is the
                # only thing preventing a torn read.
                for p_ in range(2):
                    base = rdram[0:1, 0:1]
                    nc.sync.dma_start(
                        out=rbcr[64 * p_:64 * p_ + 64, :],
                        in_=bass.AP(tensor=base.tensor,
                                    offset=base.offset + p_ * 512,
                                    ap=[[0, 64], [1, 512]]),
                    )
                for p_ in range(2):
                    h = 2 * j + p_
                    pb = 64 * p_
                    nc.vector.tensor_tensor(
                        out=zt[j][pb:pb + 64, qh * 512:(qh + 1) * 512],
                        in0=zun[pb:pb + 64, h * N + qh * 512: h * N + qh * 512 + 512],
                        in1=rbcr[pb:pb + 64, :], op=Mult,
                    )

            # ---- tiles ----
            q0 = [qkpool.tile([P, N], bf16, tag=f"qt{j}", name=f"qt_0_{j}") for j in range(NDT)]
            k0 = [qkpool.tile([P, N], bf16, tag=f"kt{j}", name=f"kt_0_{j}") for j in range(NDT)]
            q1 = [qkpool.tile([P, N], bf16, tag=f"qt{j}", name=f"qt_1_{j}") for j in range(NDT)]
            k1 = [qkpool.tile([P, N], bf16, tag=f"kt{j}", name=f"kt_1_{j}") for j in range(NDT)]

            # warm memset FIRST on the DVE queue — the big v-pad memsets
            # queue behind it, so warms unblock at ~7.3us not ~16us
            warm = wpool.tile([P, 256], bf16, tag="warm", name="warm_t")
            nc.vector.memset(warm, 0.0)

            v0 = v_pad_init(0)
            v1 = v_pad_init(1)
            a0 = alloc_attn(0)
            a1 = alloc_attn(1)

            # bf16 V mirror of head-pair 0 for slice 0 + bf16 A for the first
            # group: group (0,0,0) runs its PV in NORMAL mode. The cold-path
            # DoubleRow accumulation race poisons the first group's even
            # output rows on first execution (~20% even with DR warms);
            # non-DR for that one group sidesteps it (+1.7us PE, and the
            # global absmax error element lives in this group, so bf16 A/V
            # can only help the error).
            vb0 = vpool.tile([P, NKV * 256], bf16, tag="vb0", name="vb0")
            vz0 = vb0.rearrange("p (kv r) -> p kv r", r=256)
            nc.gpsimd.memset(vz0[:, :, 65:128], 0.0)
            nc.gpsimd.memset(vz0[:, :, 129:192], 0.0)
            nc.vector.memset(vz0[:, :, 64:65], 1.0)
            nc.vector.memset(vz0[:, :, 128:129], 1.0)
            at0 = wpool.tile([P, NPAIR * 2048], bf16, tag="at0", name="at0")

            def gen_warm(n):
                # small clock-keepers; rotate the filler psum tags so pool
                # WAR tracking stays consistent with emission order
                for _ in range(n):
                    t = _fps("warm")
                    nc.tensor.matmul(t[:, 0:256], lhsT=warm[:, 0:P], rhs=warm,
                                     start=True, stop=True)
                    yield

            # ramp the PE clock while the startup DMAs land; the follow-up
            # block after pre-attention only runs if QK inputs are late.
            # 6 (not 12): warms outrank ready QK work in priority, so every
            # extra warm delays the first S/exp by ~420ns
            for _ in gen_warm(6):
                pass

            # exercise the fp8 DoubleRow weight-load path on the cold PE
            # before the first real PV (first-execution DR corruption
            # otherwise poisons the first attention group's even rows)
            w8 = wpool.tile([P, 1024], f8, tag="w8", name="warm_f8")
            nc.vector.memset(w8, 0.0)
            for _ in range(3):
                t = _fps("drwarm")
                nc.tensor.matmul(
                    t,
                    lhsT=w8[:, 0:256].rearrange("p (sl m) -> p sl m", sl=2),
                    rhs=w8[:, :].rearrange("p (sl n) -> p sl n", sl=2),
                    start=True, stop=True, perf_mode=DR,
                )
            # Replicate the first group's exact cold-path pattern into the
            # ps_z banks: an OPEN DR accumulation group (start ... stop over
            # separate instructions) with normal-mode and 64-partition
            # matmuls interleaved — the single-instruction DR warms never
            # exercised this.
            for zi in range(2):
                zw = ps_z.tile([P, 512], f32, tag="z", name=f"zdr_{zi}")
                for acc in range(2):
                    nc.tensor.matmul(
                        zw,
                        lhsT=w8[:, 0:256].rearrange("p (sl m) -> p sl m", sl=2),
                        rhs=w8[:, :].rearrange("p (sl n) -> p sl n", sl=2),
                        start=(acc == 0), stop=(acc == 1), perf_mode=DR,
                    )
                    if acc == 0:
                        ti = _fps(f"zdr_i{zi}")
                        nc.tensor.matmul(ti[:, 0:256], lhsT=warm[0:64, 0:P],
                                         rhs=warm[0:64, :],
                                         start=True, stop=True)

            # ---- pre-attention: just enough for group (s0, j0, qh0) ----
            for _ in gen_qk_half(0, 0, 0, xt0, q0, k0):
                pass
            for kv in (0, 1):
                for _ in gen_v_chunk(0, kv, xt0, v0):
                    pass
            for _ in gen_warm(8):
                pass

            # ---- filler stream, deadline order ----
            from itertools import chain as ichain
            FILL = ichain(
                gen_qk_half(0, 0, 1, xt0, q0, k0),
                *[gen_v_chunk(0, kv, xt0, v0) for kv in range(2, NKV)],
                gen_qk_half(0, 1, 0, xt0, q0, k0),
                gen_qk_half(0, 1, 1, xt0, q0, k0),
                gen_qk_half(0, 2, 0, xt0, q0, k0),
                gen_qk_half(0, 2, 1, xt0, q0, k0),
                gen_qk_half(0, 3, 0, xt0, q0, k0),
                gen_qk_half(0, 3, 1, xt0, q0, k0),
                gen_qk_half(1, 1, 0, xt1, q1, k1),
                gen_qk_half(1, 1, 1, xt1, q1, k1),
                *[gen_v_chunk(1, kv, xt1, v1) for kv in range(NKV)],
                gen_qk_half(1, 2, 0, xt1, q1, k1),
                gen_qk_half(1, 2, 1, xt1, q1, k1),
                gen_qk_half(1, 3, 0, xt1, q1, k1),
                gen_qk_half(1, 3, 1, xt1, q1, k1),
                gen_qk_half(1, 0, 0, xt1, q1, k1),
                gen_qk_half(1, 0, 1, xt1, q1, k1),
                gen_op_chunk(0, 0, a0[1]),
                gen_op_chunk(0, 1, a0[1]),
                gen_op_chunk(0, 2, a0[1]),
                gen_op_chunk(0, 3, a0[1]),
                gen_warm(24),
            )

            # ---- flat attention pipeline ----
            group_seq = ([(0, j, qh) for j in range(NDT) for qh in range(NQH)]
                         + [(1, j, qh) for j in (1, 2, 3, 0) for qh in range(NQH)])
            steps = [(s, j, qh, kv) for (s, j, qh) in group_seq
                     for kv in range(NKV)]

            pend = []          # (due_step, fn)
            cur = {}           # per-group state: at2, zs

            for i, (s, j, qh, kv) in enumerate(steps):
                qt, kt = (q0, k0) if s == 0 else (q1, k1)
                v_sb = v0 if s == 0 else v1
                zun, zt = a0 if s == 0 else a1
                if kv == 0:
                    # zs is allocated lazily at the first (deferred) PV
                    # emission: allocating here would rotate the ps_z pool
                    # before the previous group's deferred readers (evac/
                    # stage) are emitted, breaking WAR tracking.
                    cur['zs'] = [None, None]
                first = (i < NKV)   # group (0,0,0): bf16 non-DR PV
                if kv % 2 == 0:
                    b_ = kv // 2
                    if first:
                        cur['at2'] = at0[:, b_ * 2048:(b_ + 1) * 2048]
                    else:
                        cur['at2'] = apool.tile([P, 2048], f8, tag="at",
                                                name=f"at_{s}_{j}_{qh}_{b_}")
                    # Seed at2 with 1-byte copies whose READS cover this
                    # kv-pair's V regions (even-head V at +0, odd-head V at
                    # +192 of each kv block). The PV's weights AP is
                    # invisible to the dep tracker, and a guard nop CANNOT
                    # protect a lower-priority untracked consumer (the
                    # scheduler pops the "ready" PV past the blocked nop).
                    # The PV DOES tracked-read at2, so these seeds make it
                    # transitively wait for the V data; the exps overwrite
                    # the seeded bytes before any real read.
                    for p_, off in ((0, 0), (1, 192)):
                        for q_, kvt in enumerate((2 * b_, 2 * b_ + 1)):
                            if first:
                                nc.vector.tensor_copy(
                                    cur['at2'][0:1, p_ * 1024 + q_: p_ * 1024 + q_ + 1],
                                    vb0[0:1, kvt * 256 + off: kvt * 256 + off + 1])
                            else:
                                nc.gpsimd.tensor_copy(
                                    cur['at2'][0:1, p_ * 1024 + q_: p_ * 1024 + q_ + 1],
                                    v_sb[0:1, kvt * 1024 + off: kvt * 1024 + off + 1])
                sps = ps_s.tile([P, N], f32, tag="s", name=f"s_{s}_{j}_{qh}_{kv}")
                emit_S(s, j, qh, kv, qt, kt, sps)
                emit_E(sps, cur['at2'], kv % 2)
                npop = 3 if i < 16 else 2
                for _ in range(npop):
                    next(FILL, None)
                if kv % 2 == 1:
                    b = kv // 2
                    at2, zs = cur['at2'], cur['zs']

                    def mk_pv(s=s, j=j, qh=qh, b=b, at2=at2, v_sb=v_sb,
                              zs=zs, first=first):
                        if b == 0:
                            zs[0] = ps_z.tile([P, 512], f32, tag="z",
                                              name=f"z_{s}_{j}_{qh}_0")
                            zs[1] = ps_z.tile([P, 512], f32, tag="z",
                                              name=f"z_{s}_{j}_{qh}_1")
                        if first:
                            emit_PV_bf16(b, zs)
                        else:
                            emit_PV(s, j, qh, b, at2, v_sb, zs)

                    pend.append((i + LOOK + (8 if first else 0), mk_pv))
                    if kv == NKV - 1:
                        # first group's part-B deferred extra: its zt isn't
                        # needed until the slice-0 out-proj fillers, and the
                        # spacing keeps its bounce->bcast out of the empty-
                        # queue visibility window
                        extra = 8 if i == NKV - 1 else 0
                        cell = {}
                        pend.append((i + LOOK + (8 if first else 0),
                                     (lambda s=s, j=j, qh=qh, zs=zs, zun=zun,
                                      cell=cell:
                                      emit_chain_a(s, j, qh, zs, zun, cell))))
                        pend.append((i + LOOK + extra,
                                     (lambda s=s, j=j, qh=qh, zun=zun,
                                      zt=zt, cell=cell:
                                      emit_chain_b(s, j, qh, zun, zt, cell))))
                ready = [p for p in pend if p[0] <= i]
                pend = [p for p in pend if p[0] > i]
                for _, fn in ready:
                    fn()

            for _, fn in pend:
                fn()
            for _ in FILL:   # drain leftovers (should be empty)
                pass

            # gap pool: psum-free LDWEIGHTS clock-keepers with NO deps —
            # the scheduler drops them into every PE idle gap (slice-1
            # exp-bound stretches, the tail norm chain). Lower priority
            # than all compute above, higher than the tail below.
            for _ in range(90):
                nc.tensor.ldweights(warm[:, 0:P])

            # ---- tail out-proj for slice 1 ----
            # Accumulate dd=1,2,3 for every (et,qh) while the final group's
            # norm chain lands; close with dd=0 (the last group, j=0).
            pse = {0: ps_s.tile([P, N], f32, tag="s", name="ps_tl0"),
                   1: ps_s.tile([P, N], f32, tag="s", name="ps_tl1")}
            halves = {
                2: (ps_c.tile([P, 512], f32, tag="c0", name="ps_tl2_qh0"),
                    ps_c.tile([P, 512], f32, tag="c1", name="ps_tl2_qh1")),
                3: (ps_z.tile([P, 512], f32, tag="z", name="ps_tl3_qh0"),
                    ps_z.tile([P, 512], f32, tag="z", name="ps_tl3_qh1")),
            }

            def _dst(et, qh):
                return (pse[et][:, qh * 512:(qh + 1) * 512] if et < 2
                        else halves[et][qh])

            for et in (2, 3, 0, 1):
                for dd in (1, 2, 3):
                    for qh in range(NQH):
                        nc.tensor.matmul(
                            _dst(et, qh),
                            lhsT=w_sb["wo"][:, dd * 512 + et * P: dd * 512 + (et + 1) * P],
                            rhs=a1[1][dd][:, qh * 512: qh * 512 + 512],
                            start=(dd == 1), stop=False,
                        )

            def _close(et, oeng):
                o_sb = opool.tile([P, N], bf16, tag="o", name=f"o_tl_{et}")
                for qh in range(NQH):
                    nc.tensor.matmul(
                        _dst(et, qh),
                        lhsT=w_sb["wo"][:, et * P: (et + 1) * P],
                        rhs=a1[1][0][:, qh * 512: qh * 512 + 512],
                        start=False, stop=True,
                    )
                for qh in range(NQH):
                    nc.vector.tensor_scalar_add(
                        o_sb[:, qh * 512:(qh + 1) * 512],
                        _dst(et, qh), b_sb["bo"][:, et:et + 1])
                    oeng.dma_start(
                        out=OT[1, et * P:(et + 1) * P,
                               qh * 512:(qh + 1) * 512],
                        in_=o_sb[:, qh * 512:(qh + 1) * 512])

            _close(2, nc.sync)
            _close(3, nc.scalar)
            _close(0, nc.scalar)
            _close(1, nc.sync)

    nc.compile()
    return nc


def _get_nc():
    if "nc" not in _CACHE:
        _CACHE["nc"] = _build_nc()
    return _CACHE["nc"]


def kernel(X, Wq, bq, Wk, bk, Wv, bv, Wo, bo):
    from concourse.bass_utils import run_bass_kernel_spmd

    nc = _get_nc()
    bf16 = ml_dtypes.bfloat16

    Xf = np.asarray(X, np.float32).reshape(B * T, N, D)
    XT_all = np.ascontiguousarray(Xf.transpose(0, 2, 1)).astype(bf16)  # [16, D, N]
    WQT = np.ascontiguousarray(np.asarray(Wq, np.float32).T * S_SCALE).astype(bf16)
    WKT = np.ascontiguousarray(np.asarray(Wk, np.float32).T).astype(bf16)
    WVT = np.ascontiguousarray(np.asarray(Wv, np.float32).T).astype(bf16)
    WOT = np.ascontiguousarray(np.asarray(Wo, np.float32).T).astype(bf16)
    bo_eff = (np.asarray(bo, np.float32)
              + np.asarray(Wo, np.float32) @ np.asarray(bv, np.float32))
    BQa = (np.asarray(bq, np.float32) * S_SCALE).reshape(NDT, P, 1)
    BKa = np.asarray(bk, np.float32).reshape(NDT, P, 1)
    BOa = bo_eff.reshape(NDT, P, 1)

    in_maps = []
    for c in range(NCORES):
        in_maps.append({
            "XT": np.ascontiguousarray(XT_all[c * NSLICE:(c + 1) * NSLICE]),
            "WQT": WQT, "WKT": WKT, "WVT": WVT, "WOT": WOT,
            "BQ": BQa, "BK": BKa, "BO": BOa,
        })

    trace = bool(int(os.environ.get("KERNEL_TRACE", "0")))
    kwargs = {}
    if trace:
        import tempfile
        kwargs = {"trace": True, "tmpdir": tempfile.mkdtemp(prefix="ker_trace_")}
    res = run_bass_kernel_spmd(nc, in_maps, core_ids=list(range(NCORES)), **kwargs)
    _CACHE["last_exec_ns"] = res.exec_time_ns

    out = np.empty((B * T, N, D), np.float32)
    for c in range(NCORES):
        ot = np.asarray(res.results[c]["OT"]).astype(np.float32)  # [NSLICE, D, N]
        out[c * NSLICE:(c + 1) * NSLICE] = ot.transpose(0, 2, 1)
    return out.reshape(B, T, N, D)
